# revision 12
# baseline (speedup 1.0000x reference)
"""Trainium2 Bass kernel for nn_End2EndQNetwork (8-core SPMD).

Strategy:
  - Tree/logic encoders are replicated on all 8 cores (tiny per-function
    MLPs). Nodes at each tree level are sorted by function id (host-side
    index math only), so each fid becomes a dense matmul. One-hot
    permutation matrices (host-built 0/1 constants) move level-l outputs
    into level-(l+1) sorted inputs via tensor-engine matmuls (gather +
    transpose fused).
  - q_function layer1 is row-sharded (904 rows/core, weights streamed
    from HBM as the dominant memory-bound cost), layer2 is K-sharded so
    each core produces a [128,29] column-major partial, one AllReduce
    combines them, then layer3 (scalar) is computed redundantly.
  - Output taken from core 0.
"""

import os
import numpy as np

import concourse.bacc as bacc
import concourse.bass as bass
import concourse.mybir as mybir
import concourse.tile as tile
from concourse.bass_utils import run_bass_kernel_spmd

F32 = mybir.dt.float32
AF = mybir.ActivationFunctionType
ALU = mybir.AluOpType

E = 128
NF, LF = 8, 4
T = 120
GT, OBJ, ENTS, OPS = 32, 8, 32, 8
NBLK = 113            # x vector = 113 blocks of 128
H1, H2 = 7232, 3616
NCORES = 8
RPC = H1 // NCORES    # 904 rows of W1 per core
NK2 = 8               # padded y1 chunks (904 -> 1024 = 8*128)
NM2 = 29              # ceil(3616/128)
NPAIR = 57            # 114 x-chunks paired

# q-layer weight dtype (flip to bf16 to halve HBM traffic)
import ml_dtypes
if os.environ.get("KERNEL_QF32"):
    QNP = np.float32
    QDT = F32
else:
    QNP = ml_dtypes.bfloat16
    QDT = mybir.dt.bfloat16

LAST_RESULTS = None


def _ceil32(x):
    return -(-x // 32) * 32


def _sort_plan(fids, nf):
    """Stable sort node indices by fid with each group padded to 32 cols."""
    fids = np.asarray(fids).astype(np.int64).ravel()
    n = fids.shape[0]
    counts = np.bincount(fids, minlength=nf)
    offs = np.zeros(nf, np.int64)
    cur = 0
    for f in range(nf):
        offs[f] = cur
        cur += _ceil32(int(counts[f]))
    Np = int(cur)
    col_of = np.zeros(n, np.int64)
    pos = offs.copy()
    for idx in np.argsort(fids, kind="stable"):
        f = fids[idx]
        col_of[idx] = pos[f]
        pos[f] += 1
    groups = [(int(f), int(offs[f]), int(counts[f])) for f in range(nf) if counts[f] > 0]
    return col_of, groups, Np


def _chunk_pad(nrows):
    return -(-nrows // 128) * 128


def _build_plan(inputs):
    """All host-side index math + data layout. No float arithmetic on data
    beyond layout transforms (transpose / gather / zero-pad / dtype cast)."""
    leaf_idx = np.asarray(inputs["leaf_idx"]).astype(np.int64)
    nf_fids = np.asarray(inputs["nf_fids"]).astype(np.int64)
    lf_fids = np.asarray(inputs["lf_fids"]).astype(np.int64)
    th_idx = np.asarray(inputs["th_idx"]).astype(np.int64)
    act_th_idx = np.asarray(inputs["act_th_idx"]).astype(np.int64)
    entity_emb = np.asarray(inputs["entity_emb"], dtype=np.float32)
    theorem_emb = np.asarray(inputs["theorem_emb"], dtype=np.float32)

    p = {}

    # ---- tree levels ----
    per_tree = [8, 4, 2, 1]
    offs_l = [0, 8, 12, 14]
    levels = []
    for l in range(4):
        m = per_tree[l]
        fids = nf_fids[:, offs_l[l]:offs_l[l] + m].ravel()  # flat idx t*m+j
        col_of, groups, Np = _sort_plan(fids, NF)
        nch = -(-Np // 128)
        Q = np.zeros((NF, nch * 128), np.float32)
        Q[fids, col_of] = 1.0
        levels.append(dict(m=m, fids=fids, col_of=col_of, groups=groups,
                           Np=Np, nch=nch, Q=Q))
    p["levels"] = levels

    # level-0 inputs: leaf embeddings, feature-major, fid-sorted
    lv0 = levels[0]
    x0_lo = np.zeros((128, lv0["Np"]), np.float32)
    x0_hi = np.zeros((128, lv0["Np"]), np.float32)
    x0_lo[:, lv0["col_of"]] = entity_emb[leaf_idx[:, 0::2].ravel()].T
    x0_hi[:, lv0["col_of"]] = entity_emb[leaf_idx[:, 1::2].ravel()].T
    p["x0_lo"], p["x0_hi"] = x0_lo, x0_hi

    # inter-level permutations: P_lo/P_hi [nch_l*128, Np_{l+1}]
    for l in range(3):
        a, b = levels[l], levels[l + 1]
        P_lo = np.zeros((a["nch"] * 128, b["Np"]), np.float32)
        P_hi = np.zeros((a["nch"] * 128, b["Np"]), np.float32)
        mb_ = b["m"]
        for t in range(T):
            for j in range(mb_):
                dst = b["col_of"][t * mb_ + j]
                P_lo[a["col_of"][t * 2 * mb_ + 2 * j], dst] = 1.0
                P_hi[a["col_of"][t * 2 * mb_ + 2 * j + 1], dst] = 1.0
        p[f"P{l}_lo"] = P_lo.reshape(a["nch"], 128, b["Np"])
        p[f"P{l}_hi"] = P_hi.reshape(a["nch"], 128, b["Np"])

    # logic statements (40): first 32 = GT pairs, last 8 = OBJ pairs
    col_lf, groups_lf, NLp = _sort_plan(lf_fids, LF)
    nch_lf = -(-NLp // 128)
    Qlf = np.zeros((LF, nch_lf * 128), np.float32)
    Qlf[lf_fids, col_lf] = 1.0
    p["lf"] = dict(col_of=col_lf, groups=groups_lf, Np=NLp, nch=nch_lf, Q=Qlf)

    lv3 = levels[3]
    col3 = lv3["col_of"]  # root of tree t
    P_lf_lo = np.zeros((lv3["nch"] * 128, NLp), np.float32)
    P_lf_hi = np.zeros((lv3["nch"] * 128, NLp), np.float32)
    for s in range(40):
        lt = 2 * s if s < 32 else 64 + 2 * (s - 32)
        P_lf_lo[col3[lt], col_lf[s]] = 1.0
        P_lf_hi[col3[lt + 1], col_lf[s]] = 1.0
    p["P3_lo"] = P_lf_lo.reshape(lv3["nch"], 128, NLp)
    p["P3_hi"] = P_lf_hi.reshape(lv3["nch"], 128, NLp)

    # ents(32) + ops(8) roots -> assy cols 0:40
    P_assy = np.zeros((lv3["nch"] * 128, 40), np.float32)
    for a_ in range(40):
        t = 80 + a_ if a_ < 32 else 112 + (a_ - 32)
        P_assy[col3[t], a_] = 1.0
    p["P_assy"] = P_assy.reshape(lv3["nch"], 128, 40)

    # lf-sorted -> statement order (gt 0:32 | obj 32:40)
    P_lfout = np.zeros((nch_lf * 128, 40), np.float32)
    P_lfout[col_lf, np.arange(40)] = 1.0
    p["P_lfout"] = P_lfout.reshape(nch_lf, 128, 40)

    # ---- per-function MLP weights, feature-major transposed layouts ----
    # w1T host layout [128(p), f, k(2), m(4), 128(q)] with
    # element = W1[f].T[128k+p, 128m+q] = w1[f, 128m+q, 128k+p]
    def w1_layout(w1, nf):
        A = np.asarray(w1, np.float32).reshape(nf, 4, 128, 2, 128)  # f,m,q,k,p
        return np.ascontiguousarray(A.transpose(4, 0, 3, 1, 2).reshape(128, nf * 2 * 4 * 128))

    def w2_layout(w2, nf):
        A = np.asarray(w2, np.float32).reshape(nf, 128, 4, 128)  # f,q,k,p
        return np.ascontiguousarray(A.transpose(3, 0, 2, 1).reshape(128, nf * 4 * 128))

    p["nfw1"] = w1_layout(inputs["nf_w1"], NF)
    p["nfw2"] = w2_layout(inputs["nf_w2"], NF)
    p["lfw1"] = w1_layout(inputs["lf_w1"], LF)
    p["lfw2"] = w2_layout(inputs["lf_w2"], LF)
    p["nfb1"] = np.ascontiguousarray(np.asarray(inputs["nf_b1"], np.float32))  # [8,512]
    p["nfb2"] = np.ascontiguousarray(np.asarray(inputs["nf_b2"], np.float32))  # [8,128]
    p["lfb1"] = np.ascontiguousarray(np.asarray(inputs["lf_b1"], np.float32))
    p["lfb2"] = np.ascontiguousarray(np.asarray(inputs["lf_b2"], np.float32))

    # ---- x-chunk ordering (korder) so early-ready chunks come first ----
    # reference x blocks: gt 0:32 | th 32:64 | obj 64:72 | ents 72:104 |
    #                     act 104 | ops 105:113
    korder = (list(range(32, 64)) + [104] + list(range(72, 104)) +
              list(range(105, 113)) + list(range(0, 32)) + list(range(64, 72)))
    assert len(korder) == NBLK
    p["korder"] = korder

    # theorem blocks (ready immediately): [128, 33] = th_idx cols + act col
    thact = np.zeros((128, 33), np.float32)
    thact[:, 0:32] = theorem_emb[th_idx].T
    thact[:, 32] = theorem_emb[act_th_idx[0]]
    p["thact"] = thact.astype(QNP)

    # ---- q weights, per core ----
    q_w1 = np.asarray(inputs["q_w1"], np.float32)
    q_w2 = np.asarray(inputs["q_w2"], np.float32)
    q_w3 = np.asarray(inputs["q_w3"], np.float32)

    colperm = np.concatenate([np.arange(128 * b, 128 * b + 128) for b in korder])
    w1s_cores, w2s_cores = [], []
    for c in range(NCORES):
        Wc = q_w1[c * RPC:(c + 1) * RPC][:, colperm]          # [904, 14464]
        B = np.zeros((NPAIR * 256, RPC), np.float32)
        B[:NBLK * 128] = Wc.T
        B = B.reshape(NPAIR, 2, 128, RPC).transpose(0, 2, 1, 3)
        w1s_cores.append(np.ascontiguousarray(B.reshape(NPAIR, 128, 2 * RPC)).astype(QNP))
        tmp = np.zeros((NK2 * 128, H2), np.float32)
        tmp[:RPC] = q_w2[:, c * RPC:(c + 1) * RPC].T           # [904, 3616]
        w2s_cores.append(np.ascontiguousarray(tmp.reshape(NK2, 128, H2)).astype(QNP))
    p["w1s_cores"], p["w2s_cores"] = w1s_cores, w2s_cores

    w3 = np.zeros((NM2 * 128,), np.float32)
    w3[:H2] = q_w3[0]
    p["w3T"] = np.ascontiguousarray(w3.reshape(NM2, 128).T)    # [128, 29]
    b2 = np.zeros((NM2 * 128,), np.float32)
    b2[:H2] = np.asarray(inputs["q_b2"], np.float32)
    p["b2T"] = np.ascontiguousarray(b2.reshape(NM2, 128).T)    # [128, 29]
    p["b1q"] = np.zeros((1, 1024), np.float32)
    p["b1q"][0, :H1 // NCORES] = 0.0  # placeholder; per-core b1 slice set below
    b1 = np.asarray(inputs["q_b1"], np.float32)
    p["b1q_cores"] = [np.ascontiguousarray(b1[c * RPC:(c + 1) * RPC].reshape(1, RPC))
                      for c in range(NCORES)]
    p["b3"] = np.asarray(inputs["q_b3"], np.float32).reshape(1, 1)
    p["ident"] = np.eye(128, dtype=np.float32)
    return p


def _emit_level(nc, tc, pools, lev, x_lo, x_hi, W, Q_ap, targets, ident_ap):
    """One two->one MLP level, fid-sorted feature-major columns.

    W: dict(nf, w1T(f,k,m)->AP, b1(m)->AP, w2T(f,k)->AP, b2->AP)
    targets: list of dict(P=dram AP [nch,128,Ncols], tiles=[(ps, c0, c1)])
    Level semantics per col: y = W2[f] @ relu(W1[f] @ [xlo;xhi] + b1[f]) + b2[f]
    then for each target: tgt += y_nodemajor.T selected by P.
    """
    Np, nch, groups = lev["Np"], lev["nch"], lev["groups"]
    ph, pyfm, pynm, ppool, ysb_pool, hsb_pool = (
        pools["ph"], pools["pyfm"], pools["pynm"], pools["pp"],
        pools["ysb"], pools["hsb"])

    BLK = 512
    nblocks = -(-Np // BLK)
    for bi in range(nblocks):
        b0 = bi * BLK
        b1_ = min(b0 + BLK, Np)
        bc = b1_ - b0
        h_sb = hsb_pool.tile([128, 4, bc], F32, tag="hsb")
        for m in range(4):
            h_ps = ph.tile([128, bc], F32, tag="hps")
            # bias fold: h[:, col] += b1[fid(col)]; covers all cols (start)
            nc.tensor.matmul(h_ps[:, :], W["b1"](m), Q_ap[:, b0:b1_],
                             start=True, stop=False)
            ops = []
            for (f, off, cnt) in groups:
                s = max(off, b0)
                e = min(off + _ceil32(cnt), b1_)
                if s >= e:
                    continue
                for k in range(2):
                    ops.append((f, k, s, e))
            for i, (f, k, s, e) in enumerate(ops):
                xt = x_lo if k == 0 else x_hi
                nc.tensor.matmul(h_ps[:, s - b0:e - b0], W["w1T"](f, k, m),
                                 xt[:, s:e], start=False, stop=(i == len(ops) - 1))
            nc.scalar.activation(h_sb[:, m, :], h_ps[:, :], AF.Relu)

        # per 128-chunk: matmul2 (feature-major) -> transpose -> shuffle
        for ci in range(b0 // 128, -(-b1_ // 128)):
            c0 = ci * 128
            cc = min(128, Np - c0)
            y_ps = pyfm.tile([128, 128], F32, tag="yfm")
            nc.tensor.matmul(y_ps[:, 0:cc], W["b2"](), Q_ap[:, c0:c0 + cc],
                             start=True, stop=False)
            ops = []
            for (f, off, cnt) in groups:
                s = max(off, c0)
                e = min(off + cnt, c0 + cc)
                if s >= e:
                    continue
                for k in range(4):
                    ops.append((f, k, s, e))
            for i, (f, k, s, e) in enumerate(ops):
                nc.tensor.matmul(y_ps[:, s - c0:e - c0], W["w2T"](f, k),
                                 h_sb[:, k, s - b0:e - b0],
                                 start=False, stop=(i == len(ops) - 1))
            y_sb = ysb_pool.tile([128, 128], F32, tag="ysb")
            nc.vector.tensor_copy(y_sb[:, 0:cc], y_ps[:, 0:cc])
            # node-major via PE transpose
            ynm_ps = pynm.tile([128, 128], F32, tag="ynm")
            nc.tensor.transpose(ynm_ps[0:cc, :], y_sb[:, 0:cc], ident_ap)
            ynm_sb = ysb_pool.tile([128, 128], F32, tag="ynmsb")
            nc.vector.tensor_copy(ynm_sb[0:cc, :], ynm_ps[0:cc, :])
            # shuffle into targets
            for tgt in targets:
                P_tile = ppool.tile([128, tgt["ncols"]], F32, tag="ptile")
                nc.sync.dma_start(out=P_tile[:, :], in_=tgt["P"][ci])
                for (ps, s0, s1) in tgt["tiles"]:
                    nc.tensor.matmul(ps[:, :], ynm_sb[0:cc, :],
                                     P_tile[0:cc, s0:s1],
                                     start=(ci == 0), stop=(ci == nch - 1))


def _splits(ncols):
    ns = -(-ncols // 512)
    out = []
    c = 0
    base = _ceil32(-(-ncols // ns))
    while c < ncols:
        e = min(c + base, ncols)
        out.append((c, e))
        c = e
    return out


def build_program(plan):
    nc = bacc.Bacc("TRN2", target_bir_lowering=False, debug=False,
                   num_devices=NCORES)
    levels = plan["levels"]
    lf = plan["lf"]

    def din(name, arr):
        t = nc.dram_tensor(name, list(arr.shape), mybir.dt.from_np(arr.dtype),
                           kind="ExternalInput")
        return t.ap()

    d = {}
    d["x0_lo"] = din("x0_lo", plan["x0_lo"])
    d["x0_hi"] = din("x0_hi", plan["x0_hi"])
    for l in range(3):
        d[f"P{l}_lo"] = din(f"P{l}_lo", plan[f"P{l}_lo"])
        d[f"P{l}_hi"] = din(f"P{l}_hi", plan[f"P{l}_hi"])
    for nm in ("P3_lo", "P3_hi", "P_assy", "P_lfout",
               "nfw1", "nfw2", "lfw1", "lfw2",
               "nfb1", "nfb2", "lfb1", "lfb2",
               "thact", "w3T", "b2T", "b3", "ident"):
        d[nm] = din(nm, plan[nm])
    for l in range(4):
        d[f"Q{l}"] = din(f"Q{l}", levels[l]["Q"])
    d["Qlf"] = din("Qlf", lf["Q"])
    d["w1s"] = din("w1s", plan["w1s_cores"][0])
    d["w2s"] = din("w2s", plan["w2s_cores"][0])
    d["b1q"] = din("b1q", plan["b1q_cores"][0])
    out_dram = nc.dram_tensor("out", [1, 1], F32, kind="ExternalOutput").ap()

    with tile.TileContext(nc) as tc:
        with (
            tc.tile_pool(name="const", bufs=1) as constp,
            tc.tile_pool(name="xp", bufs=2) as xp,
            tc.tile_pool(name="hsb", bufs=2) as hsbp,
            tc.tile_pool(name="ysb", bufs=3) as ysbp,
            tc.tile_pool(name="pp", bufs=3) as ppp,
            tc.tile_pool(name="w1p", bufs=3) as w1p,
            tc.tile_pool(name="w2p", bufs=2) as w2p,
            tc.tile_pool(name="dram", bufs=1, space="DRAM") as dramp,
        ):
            # ---------- constants ----------
            def load_const(nm, shape, dt=F32):
                t = constp.tile(list(shape), dt, tag=nm)
                nc.sync.dma_start(out=t[:], in_=d[nm])
                return t

            nfw1 = load_const("nfw1", plan["nfw1"].shape)
            nfw2 = load_const("nfw2", plan["nfw2"].shape)
            lfw1 = load_const("lfw1", plan["lfw1"].shape)
            lfw2 = load_const("lfw2", plan["lfw2"].shape)
            nfb1 = load_const("nfb1", (NF, 512))
            nfb2 = load_const("nfb2", (NF, 128))
            lfb1 = load_const("lfb1", (LF, 512))
            lfb2 = load_const("lfb2", (LF, 128))
            ident = load_const("ident", (128, 128))
            w3T = load_const("w3T", (128, NM2))
            b2T = load_const("b2T", (128, NM2))
            b3 = load_const("b3", (1, 1))
            b1q = load_const("b1q", (1, RPC))
            Qs = [load_const(f"Q{l}", levels[l]["Q"].shape) for l in range(4)]
            Qlf_sb = load_const("Qlf", lf["Q"].shape)
            z8 = constp.tile([NF, 128], F32, tag="z8")
            nc.vector.memset(z8[:], 0.0)

            def W_nf():
                return dict(
                    nf=NF,
                    w1T=lambda f, k, m: nfw1[:, ((f * 2 + k) * 4 + m) * 128:
                                             ((f * 2 + k) * 4 + m) * 128 + 128],
                    b1=lambda m: nfb1[:, m * 128:(m + 1) * 128],
                    w2T=lambda f, k: nfw2[:, (f * 4 + k) * 128:
                                          (f * 4 + k) * 128 + 128],
                    b2=lambda: nfb2[:, :],
                )

            W_lf = dict(
                nf=LF,
                w1T=lambda f, k, m: lfw1[:, ((f * 2 + k) * 4 + m) * 128:
                                         ((f * 2 + k) * 4 + m) * 128 + 128],
                b1=lambda m: lfb1[:, m * 128:(m + 1) * 128],
                w2T=lambda f, k: lfw2[:, (f * 4 + k) * 128:
                                      (f * 4 + k) * 128 + 128],
                b2=lambda: lfb2[:, :],
            )

            # x pieces for q-layer (korder layout), written as they become ready
            xq_th = constp.tile([128, 33], QDT, tag="xq_th")
            nc.sync.dma_start(out=xq_th[:], in_=d["thact"])
            xq_ents = constp.tile([128, 32], QDT, tag="xq_ents")
            xq_ops = constp.tile([128, 8], QDT, tag="xq_ops")
            xq_gt = constp.tile([128, 32], QDT, tag="xq_gt")
            xq_obj = constp.tile([128, 8], QDT, tag="xq_obj")
            xq_pad = constp.tile([128, 1], QDT, tag="xq_pad")
            nc.vector.memset(xq_pad[:], 0.0)

            # ---------- tree levels ----------
            x_lo = xp.tile([128, levels[0]["Np"]], F32, tag="xlo")
            x_hi = xp.tile([128, levels[0]["Np"]], F32, tag="xhi")
            nc.sync.dma_start(out=x_lo[:], in_=d["x0_lo"])
            nc.sync.dma_start(out=x_hi[:], in_=d["x0_hi"])

            pools = dict(hsb=hsbp, ysb=ysbp, pp=ppp)

            with (
                tc.tile_pool(name="ph", bufs=2, space="PSUM") as ph,
                tc.tile_pool(name="pyfm", bufs=1, space="PSUM") as pyfm,
                tc.tile_pool(name="pynm", bufs=1, space="PSUM") as pynm,
            ):
                pools.update(ph=ph, pyfm=pyfm, pynm=pynm)

                for l in range(3):
                    nxt = levels[l + 1]
                    sp = _splits(nxt["Np"])
                    with tc.tile_pool(name=f"ptgt{l}", bufs=1,
                                      space="PSUM") as ptgt:
                        tgts = []
                        for side, Pap in (("lo", d[f"P{l}_lo"]),
                                          ("hi", d[f"P{l}_hi"])):
                            tiles = []
                            for si, (s0, s1) in enumerate(sp):
                                ps = ptgt.tile([128, s1 - s0], F32,
                                               tag=f"tg_{side}{si}",
                                               name=f"tg{l}_{side}{si}")
                                tiles.append((ps, s0, s1))
                            tgts.append(dict(P=Pap, tiles=tiles, ncols=nxt["Np"]))
                        _emit_level(nc, tc, pools, levels[l], x_lo, x_hi,
                                    W_nf(), Qs[l], tgts, ident[:, :])
                        nx_lo = xp.tile([128, nxt["Np"]], F32, tag="xlo",
                                        name=f"xlo{l + 1}")
                        nx_hi = xp.tile([128, nxt["Np"]], F32, tag="xhi",
                                        name=f"xhi{l + 1}")
                        for tgt, xt in zip(tgts, (nx_lo, nx_hi)):
                            for (ps, s0, s1) in tgt["tiles"]:
                                nc.vector.tensor_copy(xt[:, s0:s1], ps[:, :])
                        x_lo, x_hi = nx_lo, nx_hi

                # level 3 -> lf inputs + assy (ents/ops)
                with tc.tile_pool(name="ptgt3", bufs=1, space="PSUM") as ptgt:
                    lf_lo_ps = ptgt.tile([128, lf["Np"]], F32, tag="tg_lflo")
                    lf_hi_ps = ptgt.tile([128, lf["Np"]], F32, tag="tg_lfhi")
                    assy_ps = ptgt.tile([128, 40], F32, tag="tg_assy")
                    tgts = [
                        dict(P=d["P3_lo"], tiles=[(lf_lo_ps, 0, lf["Np"])],
                             ncols=lf["Np"]),
                        dict(P=d["P3_hi"], tiles=[(lf_hi_ps, 0, lf["Np"])],
                             ncols=lf["Np"]),
                        dict(P=d["P_assy"], tiles=[(assy_ps, 0, 40)], ncols=40),
                    ]
                    _emit_level(nc, tc, pools, levels[3], x_lo, x_hi, W_nf(),
                                Qs[3], tgts, ident[:, :])
                    xl_lo = xp.tile([128, lf["Np"]], F32, tag="xlo")
                    xl_hi = xp.tile([128, lf["Np"]], F32, tag="xhi")
                    nc.vector.tensor_copy(xl_lo[:, :], lf_lo_ps[:, :])
                    nc.vector.tensor_copy(xl_hi[:, :], lf_hi_ps[:, :])
                    # ents -> xq cols 33:65 (korder), ops -> 65:73
                    nc.vector.tensor_copy(xq_ents[:, :], assy_ps[:, 0:32])
                    nc.vector.tensor_copy(xq_ops[:, :], assy_ps[:, 32:40])

                # logic level -> statement-order gt/obj with tanh
                with tc.tile_pool(name="ptgtlf", bufs=1, space="PSUM") as ptgt:
                    xa_ps = ptgt.tile([128, 40], F32, tag="tg_xa")
                    tgts = [dict(P=d["P_lfout"], tiles=[(xa_ps, 0, 40)],
                                 ncols=40)]
                    _emit_level(nc, tc, pools, lf, xl_lo, xl_hi, W_lf,
                                Qlf_sb, tgts, ident[:, :])
                    nc.scalar.activation(xq_gt[:, :], xa_ps[:, 0:32], AF.Tanh)
                    nc.scalar.activation(xq_obj[:, :], xa_ps[:, 32:40], AF.Tanh)

            # ---------- q layers ----------
            xq_map = []  # chunk j -> (tile, col)
            for j in range(33):
                xq_map.append((xq_th, j))
            for j in range(32):
                xq_map.append((xq_ents, j))
            for j in range(8):
                xq_map.append((xq_ops, j))
            for j in range(32):
                xq_map.append((xq_gt, j))
            for j in range(8):
                xq_map.append((xq_obj, j))
            xq_map.append((xq_pad, 0))
            assert len(xq_map) == 114

            with tc.tile_pool(name="qps", bufs=1, space="PSUM") as qps:
                y1_ps = [qps.tile([1, 452], F32, tag=f"y1p{j}", name=f"y1p{j}")
                         for j in range(2)]
                for kp in range(NPAIR):
                    wt = w1p.tile([128, 2 * RPC], QDT, tag="w1s")
                    nc.sync.dma_start(out=wt[:], in_=d["w1s"][kp])
                    for half in range(2):
                        k = 2 * kp + half
                        xt, col = xq_map[k]
                        for j in range(2):
                            nc.tensor.matmul(
                                y1_ps[j][:, :], xt[:, col:col + 1],
                                wt[:, half * RPC + j * 452: half * RPC + (j + 1) * 452],
                                start=(k == 0), stop=(k == 113))

                # bias + relu -> y1r [1, 1024] (pad zero), then transpose
                y1r = constp.tile([1, NK2 * 128], F32, tag="y1r")
                nc.vector.memset(y1r[:], 0.0)
                nc.vector.scalar_tensor_tensor(
                    y1r[0:1, 0:452], y1_ps[0][:, :], 1.0, b1q[0:1, 0:452],
                    op0=ALU.mult, op1=ALU.add)
                nc.vector.scalar_tensor_tensor(
                    y1r[0:1, 452:904], y1_ps[1][:, :], 1.0, b1q[0:1, 452:904],
                    op0=ALU.mult, op1=ALU.add)
                nc.scalar.activation(y1r[0:1, 0:904], y1r[0:1, 0:904], AF.Relu)

                y1T_ps = qps.tile([128, NK2], F32, tag="y1T")
                for k in range(NK2):
                    nc.tensor.transpose(y1T_ps[:, k:k + 1],
                                        y1r[0:1, 128 * k:128 * k + 128],
                                        ident[0:1, 0:1])
                y1T = constp.tile([128, NK2], QDT, tag="y1T_sb")
                nc.vector.tensor_copy(y1T[:, :], y1T_ps[:, :])

                # layer2: column-major partial [128, 29]
                y2_ps = qps.tile([128, NM2], F32, tag="y2p")
                nc.tensor.matmul(y2_ps[:, :], z8[:, 0:128], z8[:, 0:NM2],
                                 start=True, stop=False)
                for k in range(NK2):
                    wt2 = w2p.tile([128, H2], QDT, tag="w2s")
                    nc.sync.dma_start(out=wt2[:], in_=d["w2s"][k])
                    for m in range(NM2):
                        mm = min(128, H2 - 128 * m)
                        nc.tensor.matmul(y2_ps[0:mm, m:m + 1],
                                         wt2[:, 128 * m:128 * m + mm],
                                         y1T[:, k:k + 1],
                                         start=False, stop=False)
                nc.tensor.matmul(y2_ps[:, :], z8[:, 0:128], z8[:, 0:NM2],
                                 start=False, stop=True)

                y2p_sb = constp.tile([128, NM2], F32, tag="y2p_sb")
                nc.vector.tensor_copy(y2p_sb[:, :], y2_ps[:, :])

                # allreduce partials across the 8 cores
                y2p_d = dramp.tile([128, NM2], F32, tag="y2p_d")
                y2s_d = dramp.tile([128, NM2], F32, tag="y2s_d")
                nc.sync.dma_start(out=y2p_d[:], in_=y2p_sb[:])
                if os.environ.get("KERNEL_NOCC"):
                    nc.sync.dma_start(out=y2s_d[:], in_=y2p_d[:])
                else:
                    nc.gpsimd.collective_compute(
                        "AllReduce", ALU.add,
                        replica_groups=[list(range(NCORES))],
                        ins=[y2p_d.opt()], outs=[y2s_d.opt()])
                y2s = constp.tile([128, NM2], F32, tag="y2s")
                nc.sync.dma_start(out=y2s[:], in_=y2s_d[:])

                # + b2, relu, layer3
                y2r = constp.tile([128, NM2], F32, tag="y2r")
                nc.vector.scalar_tensor_tensor(y2r[:, :], y2s[:, :], 1.0,
                                               b2T[:, :], op0=ALU.mult, op1=ALU.add)
                nc.scalar.activation(y2r[:, :], y2r[:, :], AF.Relu)

                l3_ps = qps.tile([1, 1], F32, tag="l3")
                for k in range(NM2):
                    nc.tensor.matmul(l3_ps[:, :], y2r[:, k:k + 1],
                                     w3T[:, k:k + 1],
                                     start=(k == 0), stop=(k == NM2 - 1))
                out_sb = constp.tile([1, 1], F32, tag="out_sb")
                nc.vector.scalar_tensor_tensor(out_sb[:, :], l3_ps[:, :], 1.0,
                                               b3[:, :], op0=ALU.mult, op1=ALU.add)
                nc.sync.dma_start(out=out_dram, in_=out_sb[:])

    nc.compile()
    return nc


def _in_maps(plan):
    base = {}
    for nm in ("x0_lo", "x0_hi", "P3_lo", "P3_hi", "P_assy", "P_lfout",
               "nfw1", "nfw2", "lfw1", "lfw2", "nfb1", "nfb2", "lfb1", "lfb2",
               "thact", "w3T", "b2T", "b3", "ident"):
        base[nm] = plan[nm]
    for l in range(3):
        base[f"P{l}_lo"] = plan[f"P{l}_lo"]
        base[f"P{l}_hi"] = plan[f"P{l}_hi"]
    for l in range(4):
        base[f"Q{l}"] = plan["levels"][l]["Q"]
    base["Qlf"] = plan["lf"]["Q"]
    maps = []
    for c in range(NCORES):
        m = dict(base)
        m["w1s"] = plan["w1s_cores"][c]
        m["w2s"] = plan["w2s_cores"][c]
        m["b1q"] = plan["b1q_cores"][c]
        maps.append(m)
    return maps


def bench(nc, in_maps, iters=20):
    """Time steady-state 8-core execution with device-resident inputs.

    Mirrors bass2jax.run_bass_via_pjrt's multi-core path (minus donation)
    so repeated calls measure kernel execution + dispatch only.
    """
    import time
    import jax
    import concourse.mybir as mb
    from jax.sharding import Mesh, PartitionSpec, NamedSharding
    from jax.experimental.shard_map import shard_map
    from concourse import bass2jax
    from concourse.bass2jax import _bass_exec_p, install_neuronx_cc_hook

    install_neuronx_cc_hook()
    part_name = (nc.partition_id_tensor.name
                 if nc.partition_id_tensor is not None else None)
    in_names, out_names, out_avals, zero_outs = [], [], [], []
    for alloc in nc.m.functions[0].allocations:
        if not isinstance(mb.MemoryLocationSet, type) or not isinstance(
                alloc, mb.MemoryLocationSet):
            continue
        name = alloc.memorylocations[0].name
        if alloc.kind == "ExternalInput":
            if name != part_name:
                in_names.append(name)
        elif alloc.kind == "ExternalOutput":
            out_names.append(name)
            shape = tuple(alloc.tensor_shape)
            dtype = mb.dt.np(alloc.dtype)
            out_avals.append(jax.core.ShapedArray(shape, dtype))
            zero_outs.append(np.zeros(shape, dtype))
    n_params = len(in_names)
    all_names = in_names + out_names
    if part_name is not None:
        all_names = all_names + [part_name]

    def _body(*args):
        operands = list(args)
        if part_name is not None:
            operands.append(bass2jax.partition_id_tensor())
        outs = _bass_exec_p.bind(
            *operands, out_avals=tuple(out_avals), in_names=tuple(all_names),
            out_names=tuple(out_names), lowering_input_output_aliases=(),
            sim_require_finite=True, sim_require_nnan=True, nc=nc)
        return tuple(outs)

    devices = jax.devices()[:NCORES]
    mesh = Mesh(np.asarray(devices), ("core",))
    nspec = (PartitionSpec("core"),) * (n_params + len(out_names))
    fn = jax.jit(shard_map(_body, mesh=mesh, in_specs=nspec,
                           out_specs=(PartitionSpec("core"),) * len(out_names),
                           check_rep=False), keep_unused=True)
    concat_in = [np.concatenate([np.asarray(in_maps[c][nm])
                                 for c in range(NCORES)], axis=0)
                 for nm in in_names]
    concat_zeros = [np.zeros((NCORES * z.shape[0], *z.shape[1:]), z.dtype)
                    for z in zero_outs]
    sh = NamedSharding(mesh, PartitionSpec("core"))
    dev_in = [jax.device_put(a, sh) for a in concat_in]
    dev_zero = [jax.device_put(a, sh) for a in concat_zeros]
    out = fn(*dev_in, *dev_zero)  # warmup/compile
    jax.block_until_ready(out)
    times = []
    for _ in range(iters):
        t0 = time.perf_counter()
        out = fn(*dev_in, *dev_zero)
        jax.block_until_ready(out)
        times.append(time.perf_counter() - t0)
    res0 = np.asarray(out[0]).reshape(NCORES, -1)[0].reshape(1, 1)
    return res0, min(times), sorted(times)[len(times) // 2]


def kernel(**inputs):
    global LAST_RESULTS
    plan = _build_plan(inputs)
    nc = build_program(plan)
    in_maps = _in_maps(plan)

    if os.environ.get("KERNEL_SIM"):
        from concourse.bass_interp import MultiCoreSim
        sim = MultiCoreSim(nc, NCORES)
        for i in range(NCORES):
            for name, arr in in_maps[i].items():
                sim.cores[i].tensor(name)[:] = arr
        sim.simulate()
        return np.array(sim.cores[0].mem_tensor("out"), np.float32).reshape(1, 1)

    trace = bool(os.environ.get("KERNEL_TRACE"))
    try:
        res = run_bass_kernel_spmd(nc, in_maps, core_ids=list(range(NCORES)),
                                   trace=trace)
        LAST_RESULTS = res
        return np.asarray(res.results[0]["out"], np.float32).reshape(1, 1)
    except Exception:
        if os.environ.get("KERNEL_NO_SUBPROC"):
            raise
    # Device went unrecoverable (transient runtime flake). A fresh process
    # re-opens the device cleanly; retry there.
    return _kernel_subprocess(inputs)


def _kernel_subprocess(inputs, attempts=3):
    import subprocess
    import sys
    import tempfile
    import time

    kdir = os.path.dirname(os.path.abspath(__file__))
    last = None
    for attempt in range(attempts):
        time.sleep(3.0)
        with tempfile.TemporaryDirectory() as td:
            inp = os.path.join(td, "in.npz")
            outp = os.path.join(td, "out.npy")
            np.savez(inp, **inputs)
            code = (
                "import sys, numpy as np\n"
                f"sys.path.insert(0, {kdir!r})\n"
                "import kernel\n"
                f"d = dict(np.load({inp!r}, allow_pickle=False))\n"
                "out = kernel.kernel(**d)\n"
                f"np.save({outp!r}, out)\n"
            )
            env = dict(os.environ)
            env["KERNEL_NO_SUBPROC"] = "1"
            try:
                subprocess.run([sys.executable, "-c", code], check=True,
                               env=env, timeout=3600)
                return np.load(outp)
            except Exception as e:
                last = e
    raise last


# revision 13
# speedup vs baseline: 1.0415x; 1.0415x over previous
"""Trainium2 Bass kernel for nn_End2EndQNetwork (8-core SPMD).

Strategy:
  - Tree/logic encoders are replicated on all 8 cores (tiny per-function
    MLPs). Nodes at each tree level are sorted by function id (host-side
    index math only), so each fid becomes a dense matmul. One-hot
    permutation matrices (host-built 0/1 constants) move level-l outputs
    into level-(l+1) sorted inputs via tensor-engine matmuls (gather +
    transpose fused).
  - q_function layer1 is row-sharded (904 rows/core, weights streamed
    from HBM as the dominant memory-bound cost), layer2 is K-sharded so
    each core produces a [128,29] column-major partial, one AllReduce
    combines them, then layer3 (scalar) is computed redundantly.
  - Output taken from core 0.
"""

import os
import numpy as np

import concourse.bacc as bacc
import concourse.bass as bass
import concourse.mybir as mybir
import concourse.tile as tile
from concourse.bass_utils import run_bass_kernel_spmd

F32 = mybir.dt.float32
AF = mybir.ActivationFunctionType
ALU = mybir.AluOpType

E = 128
NF, LF = 8, 4
T = 120
GT, OBJ, ENTS, OPS = 32, 8, 32, 8
NBLK = 113            # x vector = 113 blocks of 128
H1, H2 = 7232, 3616
NCORES = 8
RPC = H1 // NCORES    # 904 rows of W1 per core
NK2 = 8               # padded y1 chunks (904 -> 1024 = 8*128)
NM2 = 29              # ceil(3616/128)
NPAIR = 57            # 114 x-chunks paired

# q-layer weight dtype (flip to bf16 to halve HBM traffic)
import ml_dtypes
if os.environ.get("KERNEL_QF32"):
    QNP = np.float32
    QDT = F32
else:
    QNP = ml_dtypes.bfloat16
    QDT = mybir.dt.bfloat16

LAST_RESULTS = None


def _ceil32(x):
    return -(-x // 32) * 32


def _sort_plan(fids, nf):
    """Stable sort node indices by fid with each group padded to 32 cols."""
    fids = np.asarray(fids).astype(np.int64).ravel()
    n = fids.shape[0]
    counts = np.bincount(fids, minlength=nf)
    offs = np.zeros(nf, np.int64)
    cur = 0
    for f in range(nf):
        offs[f] = cur
        cur += _ceil32(int(counts[f]))
    Np = int(cur)
    col_of = np.zeros(n, np.int64)
    pos = offs.copy()
    for idx in np.argsort(fids, kind="stable"):
        f = fids[idx]
        col_of[idx] = pos[f]
        pos[f] += 1
    groups = [(int(f), int(offs[f]), int(counts[f])) for f in range(nf) if counts[f] > 0]
    return col_of, groups, Np


def _chunk_pad(nrows):
    return -(-nrows // 128) * 128


def _build_plan(inputs):
    """All host-side index math + data layout. No float arithmetic on data
    beyond layout transforms (transpose / gather / zero-pad / dtype cast)."""
    leaf_idx = np.asarray(inputs["leaf_idx"]).astype(np.int64)
    nf_fids = np.asarray(inputs["nf_fids"]).astype(np.int64)
    lf_fids = np.asarray(inputs["lf_fids"]).astype(np.int64)
    th_idx = np.asarray(inputs["th_idx"]).astype(np.int64)
    act_th_idx = np.asarray(inputs["act_th_idx"]).astype(np.int64)
    entity_emb = np.asarray(inputs["entity_emb"], dtype=np.float32)
    theorem_emb = np.asarray(inputs["theorem_emb"], dtype=np.float32)

    p = {}

    # ---- tree levels ----
    per_tree = [8, 4, 2, 1]
    offs_l = [0, 8, 12, 14]
    levels = []
    for l in range(4):
        m = per_tree[l]
        fids = nf_fids[:, offs_l[l]:offs_l[l] + m].ravel()  # flat idx t*m+j
        col_of, groups, Np = _sort_plan(fids, NF)
        nch = -(-Np // 128)
        Q = np.zeros((NF, nch * 128), np.float32)
        Q[fids, col_of] = 1.0
        levels.append(dict(m=m, fids=fids, col_of=col_of, groups=groups,
                           Np=Np, nch=nch, Q=Q))
    p["levels"] = levels

    # level-0 inputs: leaf embeddings, feature-major, fid-sorted
    lv0 = levels[0]
    x0_lo = np.zeros((128, lv0["Np"]), np.float32)
    x0_hi = np.zeros((128, lv0["Np"]), np.float32)
    x0_lo[:, lv0["col_of"]] = entity_emb[leaf_idx[:, 0::2].ravel()].T
    x0_hi[:, lv0["col_of"]] = entity_emb[leaf_idx[:, 1::2].ravel()].T
    p["x0_lo"], p["x0_hi"] = x0_lo, x0_hi

    # inter-level permutations: P_lo/P_hi [nch_l*128, Np_{l+1}]
    for l in range(3):
        a, b = levels[l], levels[l + 1]
        P_lo = np.zeros((a["nch"] * 128, b["Np"]), np.float32)
        P_hi = np.zeros((a["nch"] * 128, b["Np"]), np.float32)
        mb_ = b["m"]
        for t in range(T):
            for j in range(mb_):
                dst = b["col_of"][t * mb_ + j]
                P_lo[a["col_of"][t * 2 * mb_ + 2 * j], dst] = 1.0
                P_hi[a["col_of"][t * 2 * mb_ + 2 * j + 1], dst] = 1.0
        p[f"P{l}_lo"] = P_lo.reshape(a["nch"], 128, b["Np"])
        p[f"P{l}_hi"] = P_hi.reshape(a["nch"], 128, b["Np"])

    # logic statements (40): first 32 = GT pairs, last 8 = OBJ pairs
    col_lf, groups_lf, NLp = _sort_plan(lf_fids, LF)
    nch_lf = -(-NLp // 128)
    Qlf = np.zeros((LF, nch_lf * 128), np.float32)
    Qlf[lf_fids, col_lf] = 1.0
    p["lf"] = dict(col_of=col_lf, groups=groups_lf, Np=NLp, nch=nch_lf, Q=Qlf)

    lv3 = levels[3]
    col3 = lv3["col_of"]  # root of tree t
    P_lf_lo = np.zeros((lv3["nch"] * 128, NLp), np.float32)
    P_lf_hi = np.zeros((lv3["nch"] * 128, NLp), np.float32)
    for s in range(40):
        lt = 2 * s if s < 32 else 64 + 2 * (s - 32)
        P_lf_lo[col3[lt], col_lf[s]] = 1.0
        P_lf_hi[col3[lt + 1], col_lf[s]] = 1.0
    p["P3_lo"] = P_lf_lo.reshape(lv3["nch"], 128, NLp)
    p["P3_hi"] = P_lf_hi.reshape(lv3["nch"], 128, NLp)

    # ents(32) + ops(8) roots -> assy cols 0:40
    P_assy = np.zeros((lv3["nch"] * 128, 40), np.float32)
    for a_ in range(40):
        t = 80 + a_ if a_ < 32 else 112 + (a_ - 32)
        P_assy[col3[t], a_] = 1.0
    p["P_assy"] = P_assy.reshape(lv3["nch"], 128, 40)

    # lf-sorted -> statement order (gt 0:32 | obj 32:40)
    P_lfout = np.zeros((nch_lf * 128, 40), np.float32)
    P_lfout[col_lf, np.arange(40)] = 1.0
    p["P_lfout"] = P_lfout.reshape(nch_lf, 128, 40)

    # ---- per-function MLP weights, feature-major transposed layouts ----
    # w1T host layout [128(p), f, k(2), m(4), 128(q)] with
    # element = W1[f].T[128k+p, 128m+q] = w1[f, 128m+q, 128k+p]
    def w1_layout(w1, nf):
        A = np.asarray(w1, np.float32).reshape(nf, 4, 128, 2, 128)  # f,m,q,k,p
        return np.ascontiguousarray(A.transpose(4, 0, 3, 1, 2).reshape(128, nf * 2 * 4 * 128))

    def w2_layout(w2, nf):
        A = np.asarray(w2, np.float32).reshape(nf, 128, 4, 128)  # f,q,k,p
        return np.ascontiguousarray(A.transpose(3, 0, 2, 1).reshape(128, nf * 4 * 128))

    p["nfw1"] = w1_layout(inputs["nf_w1"], NF)
    p["nfw2"] = w2_layout(inputs["nf_w2"], NF)
    p["lfw1"] = w1_layout(inputs["lf_w1"], LF)
    p["lfw2"] = w2_layout(inputs["lf_w2"], LF)
    p["nfb1"] = np.ascontiguousarray(np.asarray(inputs["nf_b1"], np.float32))  # [8,512]
    p["nfb2"] = np.ascontiguousarray(np.asarray(inputs["nf_b2"], np.float32))  # [8,128]
    p["lfb1"] = np.ascontiguousarray(np.asarray(inputs["lf_b1"], np.float32))
    p["lfb2"] = np.ascontiguousarray(np.asarray(inputs["lf_b2"], np.float32))

    # ---- x-chunk ordering (korder) so early-ready chunks come first ----
    # reference x blocks: gt 0:32 | th 32:64 | obj 64:72 | ents 72:104 |
    #                     act 104 | ops 105:113
    korder = (list(range(32, 64)) + [104] + list(range(72, 104)) +
              list(range(105, 113)) + list(range(0, 32)) + list(range(64, 72)))
    assert len(korder) == NBLK
    p["korder"] = korder

    # theorem blocks (ready immediately): [128, 33] = th_idx cols + act col
    thact = np.zeros((128, 33), np.float32)
    thact[:, 0:32] = theorem_emb[th_idx].T
    thact[:, 32] = theorem_emb[act_th_idx[0]]
    p["thact"] = thact.astype(QNP)

    # ---- q weights, per core ----
    q_w1 = np.asarray(inputs["q_w1"], np.float32)
    q_w2 = np.asarray(inputs["q_w2"], np.float32)
    q_w3 = np.asarray(inputs["q_w3"], np.float32)

    colperm = np.concatenate([np.arange(128 * b, 128 * b + 128) for b in korder])
    w1s_cores, w2s_cores = [], []
    for c in range(NCORES):
        Wc = q_w1[c * RPC:(c + 1) * RPC][:, colperm]          # [904, 14464]
        B = np.zeros((NPAIR * 256, RPC), np.float32)
        B[:NBLK * 128] = Wc.T
        B = B.reshape(NPAIR, 2, 128, RPC).transpose(0, 2, 1, 3)
        w1s_cores.append(np.ascontiguousarray(B.reshape(NPAIR, 128, 2 * RPC)).astype(QNP))
        tmp = np.zeros((NK2 * 128, H2), np.float32)
        tmp[:RPC] = q_w2[:, c * RPC:(c + 1) * RPC].T           # [904, 3616]
        w2s_cores.append(np.ascontiguousarray(tmp.reshape(NK2, 128, H2)).astype(QNP))
    p["w1s_cores"], p["w2s_cores"] = w1s_cores, w2s_cores

    w3 = np.zeros((NM2 * 128,), np.float32)
    w3[:H2] = q_w3[0]
    p["w3T"] = np.ascontiguousarray(w3.reshape(NM2, 128).T)    # [128, 29]
    b2 = np.zeros((NM2 * 128,), np.float32)
    b2[:H2] = np.asarray(inputs["q_b2"], np.float32)
    p["b2T"] = np.ascontiguousarray(b2.reshape(NM2, 128).T)    # [128, 29]
    p["b1q"] = np.zeros((1, 1024), np.float32)
    p["b1q"][0, :H1 // NCORES] = 0.0  # placeholder; per-core b1 slice set below
    b1 = np.asarray(inputs["q_b1"], np.float32)
    p["b1q_cores"] = [np.ascontiguousarray(b1[c * RPC:(c + 1) * RPC].reshape(1, RPC))
                      for c in range(NCORES)]
    p["b3"] = np.asarray(inputs["q_b3"], np.float32).reshape(1, 1)
    p["ident"] = np.eye(128, dtype=np.float32)
    return p


def _emit_level(nc, tc, pools, lev, x_lo, x_hi, W, Q_ap, targets, ident_ap):
    """One two->one MLP level, fid-sorted feature-major columns.

    W: dict(nf, w1T(f,k,m)->AP, b1(m)->AP, w2T(f,k)->AP, b2->AP)
    targets: list of dict(P=dram AP [nch,128,Ncols], tiles=[(ps, c0, c1)])
    Level semantics per col: y = W2[f] @ relu(W1[f] @ [xlo;xhi] + b1[f]) + b2[f]
    then for each target: tgt += y_nodemajor.T selected by P.
    """
    Np, nch, groups = lev["Np"], lev["nch"], lev["groups"]
    ph, pyfm, pynm, ppool, ysb_pool, hsb_pool = (
        pools["ph"], pools["pyfm"], pools["pynm"], pools["pp"],
        pools["ysb"], pools["hsb"])

    BLK = 512
    nblocks = -(-Np // BLK)
    for bi in range(nblocks):
        b0 = bi * BLK
        b1_ = min(b0 + BLK, Np)
        bc = b1_ - b0
        h_sb = hsb_pool.tile([128, 4, bc], F32, tag="hsb")
        for m in range(4):
            h_ps = ph.tile([128, bc], F32, tag="hps")
            # bias fold: h[:, col] += b1[fid(col)]; covers all cols (start)
            nc.tensor.matmul(h_ps[:, :], W["b1"](m), Q_ap[:, b0:b1_],
                             start=True, stop=False)
            ops = []
            for (f, off, cnt) in groups:
                s = max(off, b0)
                e = min(off + _ceil32(cnt), b1_)
                if s >= e:
                    continue
                for k in range(2):
                    ops.append((f, k, s, e))
            for i, (f, k, s, e) in enumerate(ops):
                xt = x_lo if k == 0 else x_hi
                nc.tensor.matmul(h_ps[:, s - b0:e - b0], W["w1T"](f, k, m),
                                 xt[:, s:e], start=False, stop=(i == len(ops) - 1))
            nc.scalar.activation(h_sb[:, m, :], h_ps[:, :], AF.Relu)

        # per 128-chunk: matmul2 (feature-major) -> transpose -> shuffle
        for ci in range(b0 // 128, -(-b1_ // 128)):
            c0 = ci * 128
            cc = min(128, Np - c0)
            y_ps = pyfm.tile([128, 128], F32, tag="yfm")
            nc.tensor.matmul(y_ps[:, 0:cc], W["b2"](), Q_ap[:, c0:c0 + cc],
                             start=True, stop=False)
            ops = []
            for (f, off, cnt) in groups:
                s = max(off, c0)
                e = min(off + cnt, c0 + cc)
                if s >= e:
                    continue
                for k in range(4):
                    ops.append((f, k, s, e))
            for i, (f, k, s, e) in enumerate(ops):
                nc.tensor.matmul(y_ps[:, s - c0:e - c0], W["w2T"](f, k),
                                 h_sb[:, k, s - b0:e - b0],
                                 start=False, stop=(i == len(ops) - 1))
            y_sb = ysb_pool.tile([128, 128], F32, tag="ysb")
            nc.vector.tensor_copy(y_sb[:, 0:cc], y_ps[:, 0:cc])
            # node-major via PE transpose
            ynm_ps = pynm.tile([128, 128], F32, tag="ynm")
            nc.tensor.transpose(ynm_ps[0:cc, :], y_sb[:, 0:cc], ident_ap)
            ynm_sb = ysb_pool.tile([128, 128], F32, tag="ynmsb")
            nc.vector.tensor_copy(ynm_sb[0:cc, :], ynm_ps[0:cc, :])
            # shuffle into targets
            for tgt in targets:
                P_tile = ppool.tile([128, tgt["ncols"]], F32, tag="ptile")
                nc.sync.dma_start(out=P_tile[:, :], in_=tgt["P"][ci])
                for (ps, s0, s1) in tgt["tiles"]:
                    nc.tensor.matmul(ps[:, :], ynm_sb[0:cc, :],
                                     P_tile[0:cc, s0:s1],
                                     start=(ci == 0), stop=(ci == nch - 1))


def _splits(ncols):
    ns = -(-ncols // 512)
    out = []
    c = 0
    base = _ceil32(-(-ncols // ns))
    while c < ncols:
        e = min(c + base, ncols)
        out.append((c, e))
        c = e
    return out


def build_program(plan):
    nc = bacc.Bacc("TRN2", target_bir_lowering=False, debug=False,
                   num_devices=NCORES)
    levels = plan["levels"]
    lf = plan["lf"]

    def din(name, arr):
        t = nc.dram_tensor(name, list(arr.shape), mybir.dt.from_np(arr.dtype),
                           kind="ExternalInput")
        return t.ap()

    d = {}
    d["x0_lo"] = din("x0_lo", plan["x0_lo"])
    d["x0_hi"] = din("x0_hi", plan["x0_hi"])
    for l in range(3):
        d[f"P{l}_lo"] = din(f"P{l}_lo", plan[f"P{l}_lo"])
        d[f"P{l}_hi"] = din(f"P{l}_hi", plan[f"P{l}_hi"])
    for nm in ("P3_lo", "P3_hi", "P_assy", "P_lfout",
               "nfw1", "nfw2", "lfw1", "lfw2",
               "nfb1", "nfb2", "lfb1", "lfb2",
               "thact", "w3T", "b2T", "b3", "ident"):
        d[nm] = din(nm, plan[nm])
    for l in range(4):
        d[f"Q{l}"] = din(f"Q{l}", levels[l]["Q"])
    d["Qlf"] = din("Qlf", lf["Q"])
    d["w1s"] = din("w1s", plan["w1s_cores"][0])
    d["w2s"] = din("w2s", plan["w2s_cores"][0])
    d["b1q"] = din("b1q", plan["b1q_cores"][0])
    out_dram = nc.dram_tensor("out", [1, 1], F32, kind="ExternalOutput").ap()

    with tile.TileContext(nc) as tc:
        with (
            tc.tile_pool(name="const", bufs=1) as constp,
            tc.tile_pool(name="xp", bufs=2) as xp,
            tc.tile_pool(name="hsb", bufs=2) as hsbp,
            tc.tile_pool(name="ysb", bufs=3) as ysbp,
            tc.tile_pool(name="pp", bufs=3) as ppp,
            tc.tile_pool(name="w1p", bufs=3) as w1p,
            tc.tile_pool(name="w2p", bufs=2) as w2p,
            tc.tile_pool(name="dram", bufs=1, space="DRAM") as dramp,
        ):
            # ---------- constants ----------
            def load_const(nm, shape, dt=F32):
                t = constp.tile(list(shape), dt, tag=nm)
                nc.sync.dma_start(out=t[:], in_=d[nm])
                return t

            nfw1 = load_const("nfw1", plan["nfw1"].shape)
            nfw2 = load_const("nfw2", plan["nfw2"].shape)
            lfw1 = load_const("lfw1", plan["lfw1"].shape)
            lfw2 = load_const("lfw2", plan["lfw2"].shape)
            nfb1 = load_const("nfb1", (NF, 512))
            nfb2 = load_const("nfb2", (NF, 128))
            lfb1 = load_const("lfb1", (LF, 512))
            lfb2 = load_const("lfb2", (LF, 128))
            ident = load_const("ident", (128, 128))
            w3T = load_const("w3T", (128, NM2))
            b2T = load_const("b2T", (128, NM2))
            b3 = load_const("b3", (1, 1))
            b1q = load_const("b1q", (1, RPC))
            Qs = [load_const(f"Q{l}", levels[l]["Q"].shape) for l in range(4)]
            Qlf_sb = load_const("Qlf", lf["Q"].shape)
            z8 = constp.tile([NF, 128], F32, tag="z8")
            nc.vector.memset(z8[:], 0.0)

            def W_nf():
                return dict(
                    nf=NF,
                    w1T=lambda f, k, m: nfw1[:, ((f * 2 + k) * 4 + m) * 128:
                                             ((f * 2 + k) * 4 + m) * 128 + 128],
                    b1=lambda m: nfb1[:, m * 128:(m + 1) * 128],
                    w2T=lambda f, k: nfw2[:, (f * 4 + k) * 128:
                                          (f * 4 + k) * 128 + 128],
                    b2=lambda: nfb2[:, :],
                )

            W_lf = dict(
                nf=LF,
                w1T=lambda f, k, m: lfw1[:, ((f * 2 + k) * 4 + m) * 128:
                                         ((f * 2 + k) * 4 + m) * 128 + 128],
                b1=lambda m: lfb1[:, m * 128:(m + 1) * 128],
                w2T=lambda f, k: lfw2[:, (f * 4 + k) * 128:
                                      (f * 4 + k) * 128 + 128],
                b2=lambda: lfb2[:, :],
            )

            # x pieces for q-layer (korder layout), written as they become ready
            xq_th = constp.tile([128, 33], QDT, tag="xq_th")
            nc.sync.dma_start(out=xq_th[:], in_=d["thact"])
            xq_ents = constp.tile([128, 32], QDT, tag="xq_ents")
            xq_ops = constp.tile([128, 8], QDT, tag="xq_ops")
            xq_gt = constp.tile([128, 32], QDT, tag="xq_gt")
            xq_obj = constp.tile([128, 8], QDT, tag="xq_obj")
            xq_pad = constp.tile([128, 1], QDT, tag="xq_pad")
            nc.vector.memset(xq_pad[:], 0.0)

            # ---------- tree levels ----------
            x_lo = xp.tile([128, levels[0]["Np"]], F32, tag="xlo")
            x_hi = xp.tile([128, levels[0]["Np"]], F32, tag="xhi")
            nc.sync.dma_start(out=x_lo[:], in_=d["x0_lo"])
            nc.sync.dma_start(out=x_hi[:], in_=d["x0_hi"])

            pools = dict(hsb=hsbp, ysb=ysbp, pp=ppp)

            with (
                tc.tile_pool(name="ph", bufs=2, space="PSUM") as ph,
                tc.tile_pool(name="pyfm", bufs=1, space="PSUM") as pyfm,
                tc.tile_pool(name="pynm", bufs=1, space="PSUM") as pynm,
            ):
                pools.update(ph=ph, pyfm=pyfm, pynm=pynm)

                for l in range(3):
                    nxt = levels[l + 1]
                    sp = _splits(nxt["Np"])
                    with tc.tile_pool(name=f"ptgt{l}", bufs=1,
                                      space="PSUM") as ptgt:
                        tgts = []
                        for side, Pap in (("lo", d[f"P{l}_lo"]),
                                          ("hi", d[f"P{l}_hi"])):
                            tiles = []
                            for si, (s0, s1) in enumerate(sp):
                                ps = ptgt.tile([128, s1 - s0], F32,
                                               tag=f"tg_{side}{si}",
                                               name=f"tg{l}_{side}{si}")
                                tiles.append((ps, s0, s1))
                            tgts.append(dict(P=Pap, tiles=tiles, ncols=nxt["Np"]))
                        _emit_level(nc, tc, pools, levels[l], x_lo, x_hi,
                                    W_nf(), Qs[l], tgts, ident[:, :])
                        nx_lo = xp.tile([128, nxt["Np"]], F32, tag="xlo",
                                        name=f"xlo{l + 1}")
                        nx_hi = xp.tile([128, nxt["Np"]], F32, tag="xhi",
                                        name=f"xhi{l + 1}")
                        for tgt, xt in zip(tgts, (nx_lo, nx_hi)):
                            for (ps, s0, s1) in tgt["tiles"]:
                                nc.vector.tensor_copy(xt[:, s0:s1], ps[:, :])
                        x_lo, x_hi = nx_lo, nx_hi

                # level 3 -> lf inputs + assy (ents/ops)
                with tc.tile_pool(name="ptgt3", bufs=1, space="PSUM") as ptgt:
                    lf_lo_ps = ptgt.tile([128, lf["Np"]], F32, tag="tg_lflo")
                    lf_hi_ps = ptgt.tile([128, lf["Np"]], F32, tag="tg_lfhi")
                    assy_ps = ptgt.tile([128, 40], F32, tag="tg_assy")
                    tgts = [
                        dict(P=d["P3_lo"], tiles=[(lf_lo_ps, 0, lf["Np"])],
                             ncols=lf["Np"]),
                        dict(P=d["P3_hi"], tiles=[(lf_hi_ps, 0, lf["Np"])],
                             ncols=lf["Np"]),
                        dict(P=d["P_assy"], tiles=[(assy_ps, 0, 40)], ncols=40),
                    ]
                    _emit_level(nc, tc, pools, levels[3], x_lo, x_hi, W_nf(),
                                Qs[3], tgts, ident[:, :])
                    xl_lo = xp.tile([128, lf["Np"]], F32, tag="xlo")
                    xl_hi = xp.tile([128, lf["Np"]], F32, tag="xhi")
                    nc.vector.tensor_copy(xl_lo[:, :], lf_lo_ps[:, :])
                    nc.vector.tensor_copy(xl_hi[:, :], lf_hi_ps[:, :])
                    # ents -> xq cols 33:65 (korder), ops -> 65:73
                    nc.vector.tensor_copy(xq_ents[:, :], assy_ps[:, 0:32])
                    nc.vector.tensor_copy(xq_ops[:, :], assy_ps[:, 32:40])

                # logic level -> statement-order gt/obj with tanh
                with tc.tile_pool(name="ptgtlf", bufs=1, space="PSUM") as ptgt:
                    xa_ps = ptgt.tile([128, 40], F32, tag="tg_xa")
                    tgts = [dict(P=d["P_lfout"], tiles=[(xa_ps, 0, 40)],
                                 ncols=40)]
                    _emit_level(nc, tc, pools, lf, xl_lo, xl_hi, W_lf,
                                Qlf_sb, tgts, ident[:, :])
                    nc.scalar.activation(xq_gt[:, :], xa_ps[:, 0:32], AF.Tanh)
                    nc.scalar.activation(xq_obj[:, :], xa_ps[:, 32:40], AF.Tanh)

            # ---------- q layers ----------
            xq_map = []  # chunk j -> (tile, col)
            for j in range(33):
                xq_map.append((xq_th, j))
            for j in range(32):
                xq_map.append((xq_ents, j))
            for j in range(8):
                xq_map.append((xq_ops, j))
            for j in range(32):
                xq_map.append((xq_gt, j))
            for j in range(8):
                xq_map.append((xq_obj, j))
            xq_map.append((xq_pad, 0))
            assert len(xq_map) == 114

            with tc.tile_pool(name="qps", bufs=1, space="PSUM") as qps:
                y1_ps = [qps.tile([1, 452], F32, tag=f"y1p{j}", name=f"y1p{j}")
                         for j in range(2)]
                for kp in range(NPAIR):
                    wt = w1p.tile([128, 2 * RPC], QDT, tag="w1s")
                    nc.sync.dma_start(out=wt[:], in_=d["w1s"][kp])
                    for half in range(2):
                        k = 2 * kp + half
                        xt, col = xq_map[k]
                        for j in range(2):
                            nc.tensor.matmul(
                                y1_ps[j][:, :], xt[:, col:col + 1],
                                wt[:, half * RPC + j * 452: half * RPC + (j + 1) * 452],
                                start=(k == 0), stop=(k == 113))

                # bias + relu -> y1r [1, 1024] (pad zero), then transpose
                y1r = constp.tile([1, NK2 * 128], F32, tag="y1r")
                nc.vector.memset(y1r[:], 0.0)
                nc.vector.scalar_tensor_tensor(
                    y1r[0:1, 0:452], y1_ps[0][:, :], 1.0, b1q[0:1, 0:452],
                    op0=ALU.mult, op1=ALU.add)
                nc.vector.scalar_tensor_tensor(
                    y1r[0:1, 452:904], y1_ps[1][:, :], 1.0, b1q[0:1, 452:904],
                    op0=ALU.mult, op1=ALU.add)
                nc.scalar.activation(y1r[0:1, 0:904], y1r[0:1, 0:904], AF.Relu)

                y1T_ps = qps.tile([128, NK2], F32, tag="y1T")
                for k in range(NK2):
                    nc.tensor.transpose(y1T_ps[:, k:k + 1],
                                        y1r[0:1, 128 * k:128 * k + 128],
                                        ident[0:1, 0:1])
                y1T = constp.tile([128, NK2], QDT, tag="y1T_sb")
                nc.vector.tensor_copy(y1T[:, :], y1T_ps[:, :])

                # layer2: column-major partial [128, 29]
                y2_ps = qps.tile([128, NM2], F32, tag="y2p")
                nc.tensor.matmul(y2_ps[:, :], z8[:, 0:128], z8[:, 0:NM2],
                                 start=True, stop=False)
                for k in range(NK2):
                    wt2 = w2p.tile([128, H2], QDT, tag="w2s")
                    nc.sync.dma_start(out=wt2[:], in_=d["w2s"][k])
                    for m in range(NM2):
                        mm = min(128, H2 - 128 * m)
                        nc.tensor.matmul(y2_ps[0:mm, m:m + 1],
                                         wt2[:, 128 * m:128 * m + mm],
                                         y1T[:, k:k + 1],
                                         start=False, stop=False)
                nc.tensor.matmul(y2_ps[:, :], z8[:, 0:128], z8[:, 0:NM2],
                                 start=False, stop=True)

                y2p_sb = constp.tile([128, NM2], F32, tag="y2p_sb")
                nc.vector.tensor_copy(y2p_sb[:, :], y2_ps[:, :])

                # allreduce partials across the 8 cores
                y2p_d = dramp.tile([128, NM2], F32, tag="y2p_d")
                y2s_d = dramp.tile([128, NM2], F32, tag="y2s_d")
                nc.sync.dma_start(out=y2p_d[:], in_=y2p_sb[:])
                if os.environ.get("KERNEL_NOCC"):
                    nc.sync.dma_start(out=y2s_d[:], in_=y2p_d[:])
                else:
                    nc.gpsimd.collective_compute(
                        "AllReduce", ALU.add,
                        replica_groups=[list(range(NCORES))],
                        ins=[y2p_d.opt()], outs=[y2s_d.opt()])
                y2s = constp.tile([128, NM2], F32, tag="y2s")
                nc.sync.dma_start(out=y2s[:], in_=y2s_d[:])

                # + b2, relu, layer3
                y2r = constp.tile([128, NM2], F32, tag="y2r")
                nc.vector.scalar_tensor_tensor(y2r[:, :], y2s[:, :], 1.0,
                                               b2T[:, :], op0=ALU.mult, op1=ALU.add)
                nc.scalar.activation(y2r[:, :], y2r[:, :], AF.Relu)

                l3_ps = qps.tile([1, 1], F32, tag="l3")
                for k in range(NM2):
                    nc.tensor.matmul(l3_ps[:, :], y2r[:, k:k + 1],
                                     w3T[:, k:k + 1],
                                     start=(k == 0), stop=(k == NM2 - 1))
                out_sb = constp.tile([1, 1], F32, tag="out_sb")
                nc.vector.scalar_tensor_tensor(out_sb[:, :], l3_ps[:, :], 1.0,
                                               b3[:, :], op0=ALU.mult, op1=ALU.add)
                nc.sync.dma_start(out=out_dram, in_=out_sb[:])

    nc.compile()
    return nc


def _in_maps(plan):
    base = {}
    for nm in ("x0_lo", "x0_hi", "P3_lo", "P3_hi", "P_assy", "P_lfout",
               "nfw1", "nfw2", "lfw1", "lfw2", "nfb1", "nfb2", "lfb1", "lfb2",
               "thact", "w3T", "b2T", "b3", "ident"):
        base[nm] = plan[nm]
    for l in range(3):
        base[f"P{l}_lo"] = plan[f"P{l}_lo"]
        base[f"P{l}_hi"] = plan[f"P{l}_hi"]
    for l in range(4):
        base[f"Q{l}"] = plan["levels"][l]["Q"]
    base["Qlf"] = plan["lf"]["Q"]
    maps = []
    for c in range(NCORES):
        m = dict(base)
        m["w1s"] = plan["w1s_cores"][c]
        m["w2s"] = plan["w2s_cores"][c]
        m["b1q"] = plan["b1q_cores"][c]
        maps.append(m)
    return maps


def bench(nc, in_maps, iters=20):
    """Time steady-state 8-core execution with device-resident inputs.

    Mirrors bass2jax.run_bass_via_pjrt's multi-core path (minus donation)
    so repeated calls measure kernel execution + dispatch only.
    """
    import time
    import jax
    import concourse.mybir as mb
    from jax.sharding import Mesh, PartitionSpec, NamedSharding
    from jax.experimental.shard_map import shard_map
    from concourse import bass2jax
    from concourse.bass2jax import _bass_exec_p, install_neuronx_cc_hook

    install_neuronx_cc_hook()
    part_name = (nc.partition_id_tensor.name
                 if nc.partition_id_tensor is not None else None)
    in_names, out_names, out_avals, zero_outs = [], [], [], []
    for alloc in nc.m.functions[0].allocations:
        if not isinstance(mb.MemoryLocationSet, type) or not isinstance(
                alloc, mb.MemoryLocationSet):
            continue
        name = alloc.memorylocations[0].name
        if alloc.kind == "ExternalInput":
            if name != part_name:
                in_names.append(name)
        elif alloc.kind == "ExternalOutput":
            out_names.append(name)
            shape = tuple(alloc.tensor_shape)
            dtype = mb.dt.np(alloc.dtype)
            out_avals.append(jax.core.ShapedArray(shape, dtype))
            zero_outs.append(np.zeros(shape, dtype))
    n_params = len(in_names)
    all_names = in_names + out_names
    if part_name is not None:
        all_names = all_names + [part_name]

    inner = int(os.environ.get("KERNEL_BENCH_INNER", "1"))

    def _body(*args):
        operands = list(args)
        if part_name is not None:
            operands.append(bass2jax.partition_id_tensor())
        for _ in range(inner):
            outs = _bass_exec_p.bind(
                *operands, out_avals=tuple(out_avals),
                in_names=tuple(all_names),
                out_names=tuple(out_names), lowering_input_output_aliases=(),
                sim_require_finite=True, sim_require_nnan=True, nc=nc)
        return tuple(outs)

    devices = jax.devices()[:NCORES]
    mesh = Mesh(np.asarray(devices), ("core",))
    nspec = (PartitionSpec("core"),) * (n_params + len(out_names))
    fn = jax.jit(shard_map(_body, mesh=mesh, in_specs=nspec,
                           out_specs=(PartitionSpec("core"),) * len(out_names),
                           check_rep=False), keep_unused=True)
    concat_in = [np.concatenate([np.asarray(in_maps[c][nm])
                                 for c in range(NCORES)], axis=0)
                 for nm in in_names]
    concat_zeros = [np.zeros((NCORES * z.shape[0], *z.shape[1:]), z.dtype)
                    for z in zero_outs]
    sh = NamedSharding(mesh, PartitionSpec("core"))
    dev_in = [jax.device_put(a, sh) for a in concat_in]
    dev_zero = [jax.device_put(a, sh) for a in concat_zeros]
    out = fn(*dev_in, *dev_zero)  # warmup/compile
    jax.block_until_ready(out)
    times = []
    for _ in range(iters):
        t0 = time.perf_counter()
        out = fn(*dev_in, *dev_zero)
        jax.block_until_ready(out)
        times.append(time.perf_counter() - t0)
    res0 = np.asarray(out[0]).reshape(NCORES, -1)[0].reshape(1, 1)
    return res0, min(times), sorted(times)[len(times) // 2]


def kernel(**inputs):
    global LAST_RESULTS
    plan = _build_plan(inputs)
    nc = build_program(plan)
    in_maps = _in_maps(plan)

    if os.environ.get("KERNEL_SIM"):
        from concourse.bass_interp import MultiCoreSim
        sim = MultiCoreSim(nc, NCORES)
        for i in range(NCORES):
            for name, arr in in_maps[i].items():
                sim.cores[i].tensor(name)[:] = arr
        sim.simulate()
        return np.array(sim.cores[0].mem_tensor("out"), np.float32).reshape(1, 1)

    trace = bool(os.environ.get("KERNEL_TRACE"))
    try:
        res = run_bass_kernel_spmd(nc, in_maps, core_ids=list(range(NCORES)),
                                   trace=trace)
        LAST_RESULTS = res
        return np.asarray(res.results[0]["out"], np.float32).reshape(1, 1)
    except Exception:
        if os.environ.get("KERNEL_NO_SUBPROC"):
            raise
    # Device went unrecoverable (transient runtime flake). A fresh process
    # re-opens the device cleanly; retry there.
    return _kernel_subprocess(inputs)


def _kernel_subprocess(inputs, attempts=3):
    import subprocess
    import sys
    import tempfile
    import time

    kdir = os.path.dirname(os.path.abspath(__file__))
    last = None
    for attempt in range(attempts):
        time.sleep(3.0)
        with tempfile.TemporaryDirectory() as td:
            inp = os.path.join(td, "in.npz")
            outp = os.path.join(td, "out.npy")
            np.savez(inp, **inputs)
            code = (
                "import sys, numpy as np\n"
                f"sys.path.insert(0, {kdir!r})\n"
                "import kernel\n"
                f"d = dict(np.load({inp!r}, allow_pickle=False))\n"
                "out = kernel.kernel(**d)\n"
                f"np.save({outp!r}, out)\n"
            )
            env = dict(os.environ)
            env["KERNEL_NO_SUBPROC"] = "1"
            try:
                subprocess.run([sys.executable, "-c", code], check=True,
                               env=env, timeout=3600)
                return np.load(outp)
            except Exception as e:
                last = e
    raise last
